# revision 25
# baseline (speedup 1.0000x reference)
"""Distributed multi-head-attention block kernel for Trainium2 (8 NeuronCores).

Sharding: data parallel over batch (2 groups of 4 cores), tensor parallel over
heads within a group (4 heads / 256 channels per core). The output projection
produces per-core partial sums combined with 4 chunked in-group ReduceScatters
(overlapped with compute); each core then layernorms the 4x128 rows it
receives (host reassembles the row permutation).

Per-core device program (identical on all cores, different data):
  - QKV projections from host-pre-transposed activations (bf16)
  - scores computed transposed [t, s]; two program variants:
      * causal: upper-triangle tiles skipped entirely (zeros DMA'd to the
        attn output), mask applied only on diagonal-crossing tiles via a
        post-exp multiply
      * general: mask folded in as exp(qk/8 + C*keep - C) for NPE of the 16
        key-tiles (TensorEngine identity-matmul adds C*keep into PSUM) and as
        a post-exp VectorEngine multiply for the rest
  - softmax denominators come for free from an extra ones-column appended to
    V in the context matmul; 1/sum normalization is applied to the attn tiles
    (written to DRAM transposed [t, s]; host un-transposes) and to the
    context rows before the output projection.
"""

import os
import sys

import numpy as np

for _p in ("/opt/trn_rl_repo", os.path.expanduser("~/.axon_site/_ro/trn_rl_repo")):
    if os.path.isdir(_p) and _p not in sys.path:
        sys.path.append(_p)

import concourse.bacc as bacc
import concourse.mybir as mybir
from concourse import masks, tile
from concourse.bass_utils import run_bass_kernel_spmd

B, S, D, H, DK = 2, 2048, 1024, 16, 64
P = 128
NCORES, GROUP = 8, 4
JC = 256           # projection channels per core (4 heads x 64)
NH = 4             # heads per core
NJT = JC // P      # 2 j-tiles
NKT = D // P       # 8 contraction tiles for projections
SBW = 512          # s-block width
NSB = S // SBW     # 4
NT = S // P        # 16 key tiles
ROWS = S // GROUP  # 512 rows per core for the LN chunk
NPE = 8            # general variant: key-tiles masked via PE matmul
CMASK = 1024.0     # psum += CMASK*keep; exp bias = -CMASK/8
EPS = 1e-5

BF16, F32 = mybir.dt.bfloat16, mybir.dt.float32
AF = mybir.ActivationFunctionType
ALU = mybir.AluOpType
AX = mybir.AxisListType
REPLICA_GROUPS = [[0, 1, 2, 3], [4, 5, 6, 7]]


def build_program(causal):
    nc = bacc.Bacc(None, target_bir_lowering=False, debug=False, num_devices=NCORES)
    dp = nc.declare_dram_parameter
    # activations pre-tiled on host: [sb, k-tile, partition, s] so each
    # (tensor, s-block) load is one contiguous 1MB DMA
    qt = dp("qt", [NSB, NKT, P, SBW], BF16, isOutput=False)
    kt = dp("kt", [NSB, NKT, P, SBW], BF16, isOutput=False)
    vt = dp("vt", [NSB, NKT, P, SBW], BF16, isOutput=False)
    # keep[sb, t, s] = 1 - mask[sb*SBW+s, t] -- [128, SBW] tiles contiguous
    keep = dp("keep", [NSB, S, SBW], BF16, isOutput=False)
    wqt = dp("wqt", [D, JC], BF16, isOutput=False)
    wkt = dp("wkt", [D, JC], BF16, isOutput=False)
    wvt = dp("wvt", [D, JC], BF16, isOutput=False)
    bqs = dp("bqs", [JC, 1], F32, isOutput=False)
    bks = dp("bks", [JC, 1], F32, isOutput=False)
    bvs = dp("bvs", [JC, 1], F32, isOutput=False)
    wot = dp("wot", [JC, D], BF16, isOutput=False)
    qres = dp("qres", [ROWS, D], F32, isOutput=False)  # Q rows + bo (permuted)
    gam = dp("gam", [P, D], F32, isOutput=False)       # gamma replicated
    bet = dp("bet", [P, D], F32, isOutput=False)       # beta replicated
    attn_t = dp("attn_t", [NH, NSB, S, SBW], BF16, isOutput=True)  # [h, sb, t, s]
    out_chunk = dp("out_chunk", [ROWS, D], F32, isOutput=True)

    with tile.TileContext(nc) as tc:
        with (
            tc.tile_pool(name="sb", bufs=1) as sb,
            tc.tile_pool(name="ps", bufs=1, space="PSUM") as ps,
            tc.tile_pool(name="dr", bufs=1, space="DRAM") as dr,
        ):
            # ---- constants ----
            ident = sb.tile([P, P], BF16, tag="ident", bufs=1)
            masks.make_identity(nc, ident[:])
            idsc = sb.tile([P, P], BF16, tag="idsc", bufs=1)
            nc.gpsimd.memset(idsc[:], 0.0)
            nc.gpsimd.affine_select(
                out=idsc[:], in_=idsc[:], compare_op=ALU.not_equal,
                fill=CMASK, base=0, pattern=[[-1, P]], channel_multiplier=1,
            )
            ones_col = sb.tile([1, P], BF16, tag="ones", bufs=1)
            nc.vector.memset(ones_col[:], 1.0)
            bias_mask = sb.tile([P, 1], F32, tag="bmask", bufs=1)
            nc.vector.memset(bias_mask[:], -CMASK / 8.0)
            bias_zero = sb.tile([P, 1], F32, tag="bzero", bufs=1)
            nc.vector.memset(bias_zero[:], 0.0)
            zero_at = sb.tile([P, SBW], BF16, tag="zat", bufs=1)
            nc.gpsimd.memset(zero_at[:], 0.0)

            # ---- weights / constants into SBUF ----
            wq_sb = sb.tile([P, NKT, JC], BF16, tag="wq", bufs=1)
            wk_sb = sb.tile([P, NKT, JC], BF16, tag="wk", bufs=1)
            wv_sb = sb.tile([P, NKT, JC], BF16, tag="wv", bufs=1)
            for w_sb, w_dr in ((wq_sb, wqt), (wk_sb, wkt), (wv_sb, wvt)):
                nc.gpsimd.dma_start(w_sb[:], w_dr[:].rearrange("(kt p) j -> p kt j", p=P))
            wo_sb = sb.tile([P, NJT, D], BF16, tag="wo", bufs=1)
            nc.gpsimd.dma_start(wo_sb[:], wot[:].rearrange("(jt p) d -> p jt d", p=P))
            bq_sb = sb.tile([P, NJT, 1], F32, tag="bq", bufs=1)
            bk_sb = sb.tile([P, NJT, 1], F32, tag="bk", bufs=1)
            bv_sb = sb.tile([P, NJT, 1], F32, tag="bv", bufs=1)
            for b_sb, b_dr in ((bq_sb, bqs), (bk_sb, bks), (bv_sb, bvs)):
                nc.gpsimd.dma_start(b_sb[:], b_dr[:].rearrange("(jt p) o -> p jt o", p=P))
            gam_sb = sb.tile([P, D], F32, tag="gam", bufs=1)
            nc.gpsimd.dma_start(gam_sb[:], gam[:])
            bet_sb = sb.tile([P, D], F32, tag="bet", bufs=1)
            nc.gpsimd.dma_start(bet_sb[:], bet[:])

            # ---- phase 1: projections (transposed outputs [j, s]) ----
            qT = sb.tile([P, NJT, S], BF16, tag="qT", bufs=1)
            kT = sb.tile([P, NJT, S], BF16, tag="kT", bufs=1)
            vT = sb.tile([P, NJT, S], BF16, tag="vT", bufs=1)
            for x_dr, w_sb, b_sb, xT in (
                (kt, wk_sb, bk_sb, kT),
                (qt, wq_sb, bq_sb, qT),
                (vt, wv_sb, bv_sb, vT),
            ):
                for sbk in range(NSB):
                    ssl = slice(sbk * SBW, (sbk + 1) * SBW)
                    x_sb = sb.tile([P, NKT, SBW], BF16, tag="xin", bufs=2)
                    nc.sync.dma_start(
                        x_sb[:], x_dr[sbk].rearrange("kt p s -> p kt s")
                    )
                    for jt in range(NJT):
                        pp = ps.tile([P, SBW], F32, tag="ps_a", bufs=2)
                        for k in range(NKT):
                            nc.tensor.matmul(
                                pp[:],
                                w_sb[:, k, jt * P:(jt + 1) * P],
                                x_sb[:, k, :],
                                start=(k == 0),
                                stop=(k == NKT - 1),
                            )
                        nc.vector.tensor_scalar_add(
                            xT[:, jt, ssl], pp[:], b_sb[:, jt, :]
                        )

            # ---- v in natural layout [t, j] with an appended ones column ----
            vst = []
            for t in range(NT):
                v_nat = sb.tile([P, NH * (DK + 1)], BF16, tag="vst", bufs=NT)
                vst.append(v_nat)
            for t in range(NT):
                for h in range(NH):
                    jt, jr = h // 2, (h % 2) * DK
                    tp = ps.tile([P, P], BF16, tag="ps_a", bufs=2)
                    nc.tensor.transpose(
                        tp[0:P, 0:DK],
                        vT[jr:jr + DK, jt, t * P:(t + 1) * P],
                        ident[jr:jr + DK, jr:jr + DK],
                    )
                    nc.vector.tensor_copy(
                        vst[t][:, h * (DK + 1):h * (DK + 1) + DK], tp[0:P, 0:DK]
                    )
                    nc.gpsimd.memset(
                        vst[t][:, h * (DK + 1) + DK:(h + 1) * (DK + 1)], 1.0
                    )

            # ---- phase 2: attention per (s-block, head), then that s-block's
            # output projection, reduce-scatter chunk, and layernorm ----
            ctxT = sb.tile([P, NJT, S], BF16, tag="ctxT", bufs=1)
            rs_ins, rs_outs = [], []
            for sbk in range(NSB):
                ri = dr.tile([SBW // P, D // SBW, P, SBW], BF16, tag="rsin",
                             bufs=NSB, name=f"rsin{sbk}")
                ro = dr.tile([D // SBW, P, SBW], BF16, tag="rsout",
                             bufs=NSB, name=f"rsout{sbk}")
                rs_ins.append(ri)
                rs_outs.append(ro)

            pending_ctx = [None]
            pending_at = [None]
            for sbk in range(NSB):
                ssl = slice(sbk * SBW, (sbk + 1) * SBW)
                # key tiles participating for this s-block
                nt_here = (sbk + 1) * (SBW // P) if causal else NT
                diag_lo = sbk * (SBW // P) if causal else NPE
                keep_lo = diag_lo if causal else 0
                keep_tiles = {}
                for t in range(keep_lo, nt_here):
                    kp = sb.tile([P, SBW], BF16, tag="keep",
                                 bufs=(8 if causal else 18),
                                 name=f"keep{sbk}_{t}")
                    nc.sync.dma_start(kp[:], keep[sbk, t * P:(t + 1) * P, :])
                    keep_tiles[t] = kp
                for h in range(NH):
                    jt, jr = h // 2, (h % 2) * DK
                    cps = ps.tile([P, SBW], F32, tag="ps_ctx", bufs=2)
                    em_tiles = []

                    def ctx_mm(t, h=h, cps=cps, em_tiles=em_tiles, last=None):
                        nc.tensor.matmul(
                            cps[0:DK + 1, :],
                            vst[t][:, h * (DK + 1):(h + 1) * (DK + 1)],
                            em_tiles[t][:],
                            start=(t == 0), stop=(t == last),
                        )

                    # upper-triangle attn tiles stay zero: the PJRT path
                    # donates pre-zeroed output buffers
                    LAG = 3
                    FIN_T = 2
                    FIN_T2 = 8
                    for t in range(nt_here):
                        sp = ps.tile([P, SBW], F32, tag="ps_s", bufs=4)
                        em = sb.tile([P, SBW], BF16, tag="em", bufs=2 * NT + 2)
                        if causal:
                            diag = t >= diag_lo
                            nc.tensor.matmul(
                                sp[:],
                                kT[jr:jr + DK, jt, t * P:(t + 1) * P],
                                qT[jr:jr + DK, jt, ssl],
                                start=True, stop=not diag,
                            )
                            if diag:
                                nc.tensor.matmul(
                                    sp[:], idsc[:], keep_tiles[t][:],
                                    start=False, stop=True,
                                )
                            nc.scalar.activation(
                                em[:], sp[:], AF.Exp,
                                bias=(bias_mask[:] if diag else bias_zero[:]),
                                scale=0.125,
                            )
                        elif t < NPE:
                            nc.tensor.matmul(
                                sp[:],
                                kT[jr:jr + DK, jt, t * P:(t + 1) * P],
                                qT[jr:jr + DK, jt, ssl],
                                start=True, stop=False,
                            )
                            nc.tensor.matmul(
                                sp[:], idsc[:], keep_tiles[t][:],
                                start=False, stop=True,
                            )
                            nc.scalar.activation(
                                em[:], sp[:], AF.Exp,
                                bias=bias_mask[:], scale=0.125,
                            )
                        else:
                            nc.tensor.matmul(
                                sp[:],
                                kT[jr:jr + DK, jt, t * P:(t + 1) * P],
                                qT[jr:jr + DK, jt, ssl],
                                start=True, stop=True,
                            )
                            etmp = sb.tile([P, SBW], BF16, tag="etmp", bufs=4)
                            nc.scalar.activation(
                                etmp[:], sp[:], AF.Exp,
                                bias=bias_zero[:], scale=0.125,
                            )
                            nc.vector.tensor_mul(em[:], etmp[:], keep_tiles[t][:])
                        em_tiles.append(em)
                        if t >= LAG:
                            ctx_mm(t - LAG, last=nt_here - 1)
                        if t == FIN_T and pending_ctx[0] is not None:
                            pending_ctx[0]()
                            pending_ctx[0] = None
                        if t == FIN_T2 and pending_at[0] is not None:
                            pending_at[0]()
                            pending_at[0] = None
                    for t in range(max(0, nt_here - LAG), nt_here):
                        ctx_mm(t, last=nt_here - 1)
                    if pending_ctx[0] is not None:
                        pending_ctx[0]()
                        pending_ctx[0] = None
                    if pending_at[0] is not None:
                        pending_at[0]()
                        pending_at[0] = None
                    # softmax denominators sit in cps row DK; reciprocal as
                    # exp(-ln(x)) on the ScalarEngine (same table set as the
                    # softmax exp; DVE's iterative divide costs 2.7us here).
                    # The PE broadcast + normalize are deferred into the next
                    # head's score loop so the TensorEngine never waits.
                    lrow = sb.tile([1, SBW], F32, tag="lrow", bufs=4)
                    nc.scalar.activation(
                        lrow[:], cps[DK:DK + 1, :], AF.Ln,
                        bias=bias_zero[0:1, :], scale=1.0,
                    )
                    rrow = sb.tile([1, SBW], BF16, tag="rrow", bufs=4)
                    nc.scalar.activation(
                        rrow[:], lrow[:], AF.Exp,
                        bias=bias_zero[0:1, :], scale=-1.0,
                    )

                    rbc_box = {}

                    def fin_ctx(h=h, jt=jt, jr=jr, cps=cps, rrow=rrow,
                                box=rbc_box, ssl=ssl):
                        bps = ps.tile([P, SBW], F32, tag="ps_a", bufs=2)
                        nc.tensor.matmul(
                            bps[:], ones_col[:], rrow[:], start=True, stop=True
                        )
                        rbc = sb.tile([P, SBW], BF16, tag="rbc", bufs=2)
                        nc.vector.tensor_copy(rbc[:], bps[:])
                        box["rbc"] = rbc
                        nc.vector.tensor_mul(
                            ctxT[jr:jr + DK, jt, ssl], cps[0:DK, :], rbc[0:DK, :]
                        )

                    def fin_at(h=h, box=rbc_box, em_tiles=em_tiles,
                               nt_h=nt_here, sbk=sbk):
                        rbc = box["rbc"]
                        for t in range(nt_h):
                            at = sb.tile([P, SBW], BF16, tag="at", bufs=8)
                            nc.vector.tensor_mul(at[:], em_tiles[t][:], rbc[:])
                            nc.sync.dma_start(
                                attn_t[h, sbk, t * P:(t + 1) * P, :], at[:]
                            )

                    pending_ctx[0] = fin_ctx
                    pending_at[0] = fin_at
                # flush before the output projection: ctxT must be complete,
                # but the last head's attn-normalize can wait until after the
                # out-proj instructions are queued
                if pending_ctx[0] is not None:
                    pending_ctx[0]()
                    pending_ctx[0] = None
                last_at = pending_at[0]
                pending_at[0] = None
                if sbk < NSB - 1 and last_at is not None:
                    last_at()
                    last_at = None
                # ---- output projection for this s-block's row tiles ----
                for sti in range(SBW // P):
                    st = sbk * (SBW // P) + sti
                    for ob in range(D // SBW):
                        ops = ps.tile([P, SBW], F32, tag="ps_a", bufs=2)
                        for jt in range(NJT):
                            nc.tensor.matmul(
                                ops[:],
                                ctxT[:, jt, st * P:(st + 1) * P],
                                wo_sb[:, jt, ob * SBW:(ob + 1) * SBW],
                                start=(jt == 0), stop=(jt == NJT - 1),
                            )
                        ob_sb = sb.tile([P, SBW], BF16, tag="ob", bufs=4)
                        nc.vector.tensor_copy(ob_sb[:], ops[:])
                        nc.sync.dma_start(rs_ins[sbk][sti, ob], ob_sb[:])
                if last_at is not None:
                    last_at()
                # ---- reduce-scatter this chunk; rank r receives the reduced
                # rows [sbk*512 + r*128, +128) of this batch's output ----
                nc.gpsimd.collective_compute(
                    "ReduceScatter", ALU.add, replica_groups=REPLICA_GROUPS,
                    ins=[rs_ins[sbk].opt()], outs=[rs_outs[sbk].opt()],
                )
                # ---- residual + layernorm on the received 128 rows ----
                rsl = slice(sbk * P, (sbk + 1) * P)
                xr = sb.tile([P, D], BF16, tag="xr", bufs=2)
                for ob in range(D // SBW):
                    nc.sync.dma_start(
                        xr[:, ob * SBW:(ob + 1) * SBW], rs_outs[sbk][ob]
                    )
                qr = sb.tile([P, D], F32, tag="qr", bufs=2)
                nc.gpsimd.dma_start(qr[:], qres[rsl, :])
                x = sb.tile([P, D], F32, tag="x", bufs=2)
                nc.vector.tensor_add(x[:], xr[:], qr[:])
                stat = sb.tile([P, 8], F32, tag="stat", bufs=4)
                nc.vector.tensor_reduce(stat[:, 0:1], x[:], axis=AX.X, op=ALU.add)
                nc.vector.tensor_scalar_mul(stat[:, 1:2], stat[:, 0:1], -1.0 / D)
                xc = sb.tile([P, D], F32, tag="xc", bufs=2)
                nc.vector.tensor_scalar_add(xc[:], x[:], stat[:, 1:2])
                sqd = sb.tile([P, D], BF16, tag="sqd", bufs=2)
                nc.scalar.activation(sqd[:], xc[:], AF.Square, accum_out=stat[:, 2:3])
                nc.vector.tensor_scalar(
                    stat[:, 3:4], stat[:, 2:3], 1.0 / D, EPS, ALU.mult, ALU.add
                )
                # 1/sqrt(v) = exp(-0.5*ln(v)) -- stays in the exp/ln table set
                nc.scalar.activation(
                    stat[:, 4:5], stat[:, 3:4], AF.Ln, bias=bias_zero[:], scale=1.0
                )
                nc.scalar.activation(
                    stat[:, 5:6], stat[:, 4:5], AF.Exp, bias=bias_zero[:], scale=-0.5
                )
                y = sb.tile([P, D], F32, tag="y", bufs=2)
                nc.vector.scalar_tensor_tensor(
                    y[:], xc[:], stat[:, 5:6], gam_sb[:], op0=ALU.mult, op1=ALU.mult
                )
                nc.vector.tensor_add(y[:], y[:], bet_sb[:])
                nc.sync.dma_start(out_chunk[rsl, :], y[:])

    nc.compile()
    return nc


_PROGRAMS = {}


def _get_program(causal):
    if causal not in _PROGRAMS:
        _PROGRAMS[causal] = build_program(causal)
    return _PROGRAMS[causal]


def _core_rows(r):
    """Global row indices (within a batch) that core with group-rank r
    layernorms, in out_chunk order: 4 pieces of 128 rows, piece sbk holding
    rows [sbk*512 + r*128, +128)."""
    idx = []
    for sbk in range(NSB):
        start = sbk * SBW + r * P
        idx.extend(range(start, start + P))
    return np.asarray(idx)


def _build_in_maps(Q, K, V, attn_mask, Wq, bq, Wk, bk, Wv, bv, Wo, bo, gamma, beta):
    bf16 = mybir.dt.np(BF16)
    f32 = np.float32
    Q = np.asarray(Q, f32)
    K = np.asarray(K, f32)
    V = np.asarray(V, f32)
    mask = np.asarray(attn_mask, bool)
    Wq, Wk, Wv, Wo = (np.asarray(w, f32) for w in (Wq, Wk, Wv, Wo))
    bq, bk, bv, bo = (np.asarray(b_, f32) for b_ in (bq, bk, bv, bo))
    gamma = np.asarray(gamma, f32)
    beta = np.asarray(beta, f32)

    causal = bool(
        np.array_equal(
            mask, np.broadcast_to(np.triu(np.ones((S, S), bool), 1), (B, S, S))
        )
    )

    def _tile_xt(x):
        # [S, D] -> XT [D, S] -> [NSB, NKT, P, SBW]
        xt = x.T.astype(bf16)
        return np.ascontiguousarray(
            xt.reshape(NKT, P, NSB, SBW).transpose(2, 0, 1, 3)
        )

    qt_b = [_tile_xt(Q[b]) for b in range(B)]
    kt_b = [_tile_xt(K[b]) for b in range(B)]
    vt_b = [_tile_xt(V[b]) for b in range(B)]
    # keep[sb, t, s] = (~mask[b])[sb*SBW+s, t]
    keep_b = [
        np.ascontiguousarray(
            (~mask[b]).T.astype(bf16).reshape(S, NSB, SBW).transpose(1, 0, 2)
        )
        for b in range(B)
    ]
    gam_r = np.ascontiguousarray(np.broadcast_to(gamma, (P, D))).astype(f32)
    bet_r = np.ascontiguousarray(np.broadcast_to(beta, (P, D))).astype(f32)

    in_maps = []
    for c in range(NCORES):
        b, r = c // GROUP, c % GROUP
        j0 = r * JC
        rows = _core_rows(r)
        in_maps.append({
            "qt": qt_b[b], "kt": kt_b[b], "vt": vt_b[b], "keep": keep_b[b],
            "wqt": Wq[j0:j0 + JC, :].T.astype(bf16),
            "wkt": Wk[j0:j0 + JC, :].T.astype(bf16),
            "wvt": Wv[j0:j0 + JC, :].T.astype(bf16),
            "bqs": bq[j0:j0 + JC].reshape(JC, 1).astype(f32),
            "bks": bk[j0:j0 + JC].reshape(JC, 1).astype(f32),
            "bvs": bv[j0:j0 + JC].reshape(JC, 1).astype(f32),
            "wot": Wo[:, j0:j0 + JC].T.astype(bf16),
            "qres": (Q[b, rows, :] + bo).astype(f32),
            "gam": gam_r, "bet": bet_r,
        })
    return in_maps, causal


def _assemble(results):
    f32 = np.float32
    out = np.empty((B, S, D), f32)
    attn = np.empty((B, H, S, S), f32)
    for c in range(NCORES):
        b, r = c // GROUP, c % GROUP
        out[b, _core_rows(r)] = np.asarray(results[c]["out_chunk"], f32)
        a = np.asarray(results[c]["attn_t"]).astype(f32)  # [NH, sb, t, s]
        # attn[b, h, sb*SBW+s, t] = a[hl, sb, t, s]
        attn[b, r * NH:(r + 1) * NH] = (
            a.transpose(0, 1, 3, 2).reshape(NH, S, S)
        )
    return out, attn


def _run(in_maps, causal, trace=False, **kwargs):
    nc = _get_program(causal)
    return run_bass_kernel_spmd(
        nc, in_maps, core_ids=list(range(NCORES)), trace=trace, **kwargs
    )


def kernel(**inputs):
    in_maps, causal = _build_in_maps(**inputs)
    res = _run(in_maps, causal, trace=False)
    return _assemble(res.results)


def _enable_axon_ntff_hook():
    """Restore the NTFF profile hook that trn_boot could not register
    (antenv.axon_hooks is absent from this image) and skip the shared-bucket
    artifact upload. Only used by the local profiling harness."""
    import types

    import antenv
    import concourse.bass_utils as bu

    if "antenv.axon_hooks" not in sys.modules:
        mod = types.ModuleType("antenv.axon_hooks")
        mod._hook = None
        mod.set_axon_ntff_profile_hook = lambda h: setattr(mod, "_hook", h)
        mod.get_axon_ntff_profile_hook = lambda: mod._hook
        sys.modules["antenv.axon_hooks"] = mod
        antenv.axon_hooks = mod
        from trn_agent_boot.trn_boot import _ntff_profile_via_ctypes

        mod._hook = _ntff_profile_via_ctypes("/opt/axon/libaxon_pjrt.so")
    bu.upload_artifacts = lambda tmpdir: tmpdir


def kernel_profiled(tmpdir=None, **inputs):
    """Like kernel() but also returns neuron-profile exec time in ns."""
    _enable_axon_ntff_hook()
    in_maps, causal = _build_in_maps(**inputs)
    res = _run(in_maps, causal, trace=True, tmpdir=tmpdir)
    return _assemble(res.results), res.exec_time_ns


# revision 26
# speedup vs baseline: 1.0331x; 1.0331x over previous
"""Distributed multi-head-attention block kernel for Trainium2 (8 NeuronCores).

Sharding: data parallel over batch (2 groups of 4 cores), tensor parallel over
heads within a group (4 heads / 256 channels per core). The output projection
produces per-core partial sums combined with 4 chunked in-group ReduceScatters
(overlapped with compute); each core then layernorms the 4x128 rows it
receives (host reassembles the row permutation).

Per-core device program (identical on all cores, different data):
  - QKV projections from host-pre-transposed activations (bf16)
  - scores computed transposed [t, s]; two program variants:
      * causal: upper-triangle tiles skipped entirely (zeros DMA'd to the
        attn output), mask applied only on diagonal-crossing tiles via a
        post-exp multiply
      * general: mask folded in as exp(qk/8 + C*keep - C) for NPE of the 16
        key-tiles (TensorEngine identity-matmul adds C*keep into PSUM) and as
        a post-exp VectorEngine multiply for the rest
  - softmax denominators come for free from an extra ones-column appended to
    V in the context matmul; 1/sum normalization is applied to the attn tiles
    (written to DRAM transposed [t, s]; host un-transposes) and to the
    context rows before the output projection.
"""

import os
import sys

import numpy as np

for _p in ("/opt/trn_rl_repo", os.path.expanduser("~/.axon_site/_ro/trn_rl_repo")):
    if os.path.isdir(_p) and _p not in sys.path:
        sys.path.append(_p)

import concourse.bacc as bacc
import concourse.mybir as mybir
from concourse import masks, tile
from concourse.bass_utils import run_bass_kernel_spmd

B, S, D, H, DK = 2, 2048, 1024, 16, 64
P = 128
NCORES, GROUP = 8, 4
JC = 256           # projection channels per core (4 heads x 64)
NH = 4             # heads per core
NJT = JC // P      # 2 j-tiles
NKT = D // P       # 8 contraction tiles for projections
SBW = 512          # s-block width
NSB = S // SBW     # 4
NT = S // P        # 16 key tiles
ROWS = S // GROUP  # 512 rows per core for the LN chunk
NPE = 8            # general variant: key-tiles masked via PE matmul
CMASK = 1024.0     # psum += CMASK*keep; exp bias = -CMASK/8
EPS = 1e-5

BF16, F32 = mybir.dt.bfloat16, mybir.dt.float32
AF = mybir.ActivationFunctionType
ALU = mybir.AluOpType
AX = mybir.AxisListType
REPLICA_GROUPS = [[0, 1, 2, 3], [4, 5, 6, 7]]


def build_program(causal):
    nc = bacc.Bacc(None, target_bir_lowering=False, debug=False, num_devices=NCORES)
    dp = nc.declare_dram_parameter
    # activations pre-tiled on host: [sb, k-tile, partition, s] so each
    # (tensor, s-block) load is one contiguous 1MB DMA
    qt = dp("qt", [NSB, NKT, P, SBW], BF16, isOutput=False)
    kt = dp("kt", [NSB, NKT, P, SBW], BF16, isOutput=False)
    vt = dp("vt", [NSB, NKT, P, SBW], BF16, isOutput=False)
    # keep[sb, t, s] = 1 - mask[sb*SBW+s, t] -- [128, SBW] tiles contiguous
    keep = dp("keep", [NSB, S, SBW], BF16, isOutput=False)
    wqt = dp("wqt", [D, JC], BF16, isOutput=False)
    wkt = dp("wkt", [D, JC], BF16, isOutput=False)
    wvt = dp("wvt", [D, JC], BF16, isOutput=False)
    bqs = dp("bqs", [JC, 1], F32, isOutput=False)
    bks = dp("bks", [JC, 1], F32, isOutput=False)
    bvs = dp("bvs", [JC, 1], F32, isOutput=False)
    wot = dp("wot", [JC, D], BF16, isOutput=False)
    qres = dp("qres", [ROWS, D], F32, isOutput=False)  # Q rows + bo (permuted)
    gam = dp("gam", [P, D], F32, isOutput=False)       # gamma replicated
    bet = dp("bet", [P, D], F32, isOutput=False)       # beta replicated
    attn_t = dp("attn_t", [NH, NSB, S, SBW], BF16, isOutput=True)  # [h, sb, t, s]
    out_chunk = dp("out_chunk", [ROWS, D], F32, isOutput=True)

    with tile.TileContext(nc) as tc:
        with (
            tc.tile_pool(name="sb", bufs=1) as sb,
            tc.tile_pool(name="ps", bufs=1, space="PSUM") as ps,
            tc.tile_pool(name="dr", bufs=1, space="DRAM") as dr,
        ):
            # ---- constants ----
            ident = sb.tile([P, P], BF16, tag="ident", bufs=1)
            masks.make_identity(nc, ident[:])
            idsc = sb.tile([P, P], BF16, tag="idsc", bufs=1)
            nc.gpsimd.memset(idsc[:], 0.0)
            nc.gpsimd.affine_select(
                out=idsc[:], in_=idsc[:], compare_op=ALU.not_equal,
                fill=CMASK, base=0, pattern=[[-1, P]], channel_multiplier=1,
            )
            ones_col = sb.tile([1, P], BF16, tag="ones", bufs=1)
            nc.vector.memset(ones_col[:], 1.0)
            bias_mask = sb.tile([P, 1], F32, tag="bmask", bufs=1)
            nc.vector.memset(bias_mask[:], -CMASK / 8.0)
            bias_zero = sb.tile([P, 1], F32, tag="bzero", bufs=1)
            nc.vector.memset(bias_zero[:], 0.0)
            zero_at = sb.tile([P, SBW], BF16, tag="zat", bufs=1)
            nc.gpsimd.memset(zero_at[:], 0.0)

            # ---- weights / constants into SBUF ----
            wq_sb = sb.tile([P, NKT, JC], BF16, tag="wq", bufs=1)
            wk_sb = sb.tile([P, NKT, JC], BF16, tag="wk", bufs=1)
            wv_sb = sb.tile([P, NKT, JC], BF16, tag="wv", bufs=1)
            for w_sb, w_dr in ((wq_sb, wqt), (wk_sb, wkt), (wv_sb, wvt)):
                nc.gpsimd.dma_start(w_sb[:], w_dr[:].rearrange("(kt p) j -> p kt j", p=P))
            wo_sb = sb.tile([P, NJT, D], BF16, tag="wo", bufs=1)
            nc.gpsimd.dma_start(wo_sb[:], wot[:].rearrange("(jt p) d -> p jt d", p=P))
            bq_sb = sb.tile([P, NJT, 1], F32, tag="bq", bufs=1)
            bk_sb = sb.tile([P, NJT, 1], F32, tag="bk", bufs=1)
            bv_sb = sb.tile([P, NJT, 1], F32, tag="bv", bufs=1)
            for b_sb, b_dr in ((bq_sb, bqs), (bk_sb, bks), (bv_sb, bvs)):
                nc.gpsimd.dma_start(b_sb[:], b_dr[:].rearrange("(jt p) o -> p jt o", p=P))
            gam_sb = sb.tile([P, D], F32, tag="gam", bufs=1)
            nc.gpsimd.dma_start(gam_sb[:], gam[:])
            bet_sb = sb.tile([P, D], F32, tag="bet", bufs=1)
            nc.gpsimd.dma_start(bet_sb[:], bet[:])

            # ---- phase 1: projections (transposed outputs [j, s]) ----
            qT = sb.tile([P, NJT, S], BF16, tag="qT", bufs=1)
            kT = sb.tile([P, NJT, S], BF16, tag="kT", bufs=1)
            vT = sb.tile([P, NJT, S], BF16, tag="vT", bufs=1)
            for x_dr, w_sb, b_sb, xT in (
                (kt, wk_sb, bk_sb, kT),
                (qt, wq_sb, bq_sb, qT),
                (vt, wv_sb, bv_sb, vT),
            ):
                for sbk in range(NSB):
                    ssl = slice(sbk * SBW, (sbk + 1) * SBW)
                    x_sb = sb.tile([P, NKT, SBW], BF16, tag="xin", bufs=2)
                    nc.sync.dma_start(
                        x_sb[:], x_dr[sbk].rearrange("kt p s -> p kt s")
                    )
                    for jt in range(NJT):
                        pp = ps.tile([P, SBW], F32, tag="ps_a", bufs=2)
                        for k in range(NKT):
                            nc.tensor.matmul(
                                pp[:],
                                w_sb[:, k, jt * P:(jt + 1) * P],
                                x_sb[:, k, :],
                                start=(k == 0),
                                stop=(k == NKT - 1),
                            )
                        nc.vector.tensor_scalar_add(
                            xT[:, jt, ssl], pp[:], b_sb[:, jt, :]
                        )

            # ---- v in natural layout [t, j] with an appended ones column ----
            vst = []
            for t in range(NT):
                v_nat = sb.tile([P, NH * (DK + 1)], BF16, tag="vst", bufs=NT)
                vst.append(v_nat)
            for t in range(NT):
                for h in range(NH):
                    jt, jr = h // 2, (h % 2) * DK
                    tp = ps.tile([P, P], BF16, tag="ps_a", bufs=2)
                    nc.tensor.transpose(
                        tp[0:P, 0:DK],
                        vT[jr:jr + DK, jt, t * P:(t + 1) * P],
                        ident[jr:jr + DK, jr:jr + DK],
                    )
                    nc.vector.tensor_copy(
                        vst[t][:, h * (DK + 1):h * (DK + 1) + DK], tp[0:P, 0:DK]
                    )
                    nc.gpsimd.memset(
                        vst[t][:, h * (DK + 1) + DK:(h + 1) * (DK + 1)], 1.0
                    )

            # ---- phase 2: attention per (s-block, head), then that s-block's
            # output projection, reduce-scatter chunk, and layernorm ----
            ctxT = sb.tile([P, NJT, S], BF16, tag="ctxT", bufs=1)
            rs_ins, rs_outs = [], []
            for sbk in range(NSB):
                ri = dr.tile([SBW // P, D // SBW, P, SBW], BF16, tag="rsin",
                             bufs=NSB, name=f"rsin{sbk}")
                ro = dr.tile([D // SBW, P, SBW], BF16, tag="rsout",
                             bufs=NSB, name=f"rsout{sbk}")
                rs_ins.append(ri)
                rs_outs.append(ro)

            pending_ctx = [None]
            pending_at = [None]
            for sbk in range(NSB):
                ssl = slice(sbk * SBW, (sbk + 1) * SBW)
                # key tiles participating for this s-block
                nt_here = (sbk + 1) * (SBW // P) if causal else NT
                diag_lo = sbk * (SBW // P) if causal else NPE
                keep_lo = diag_lo if causal else 0
                keep_tiles = {}
                for t in range(keep_lo, nt_here):
                    kp = sb.tile([P, SBW], BF16, tag="keep",
                                 bufs=(8 if causal else 18),
                                 name=f"keep{sbk}_{t}")
                    nc.sync.dma_start(kp[:], keep[sbk, t * P:(t + 1) * P, :])
                    keep_tiles[t] = kp
                for h in range(NH):
                    jt, jr = h // 2, (h % 2) * DK
                    cps = ps.tile([P, SBW], F32, tag="ps_ctx", bufs=2)
                    em_tiles = []

                    def ctx_mm(t, h=h, cps=cps, em_tiles=em_tiles, last=None):
                        nc.tensor.matmul(
                            cps[0:DK + 1, :],
                            vst[t][:, h * (DK + 1):(h + 1) * (DK + 1)],
                            em_tiles[t][:],
                            start=(t == 0), stop=(t == last),
                        )

                    # upper-triangle attn tiles stay zero: the PJRT path
                    # donates pre-zeroed output buffers
                    LAG = 3
                    FIN_T = 2
                    FIN_T2 = 8
                    for t in range(nt_here):
                        sp = ps.tile([P, SBW], F32, tag="ps_s", bufs=4)
                        em = sb.tile([P, SBW], BF16, tag="em",
                                     bufs=(44 if causal else 2 * NT + 2))
                        if causal:
                            diag = t >= diag_lo
                            nc.tensor.matmul(
                                sp[:],
                                kT[jr:jr + DK, jt, t * P:(t + 1) * P],
                                qT[jr:jr + DK, jt, ssl],
                                start=True, stop=not diag,
                            )
                            if diag:
                                nc.tensor.matmul(
                                    sp[:], idsc[:], keep_tiles[t][:],
                                    start=False, stop=True,
                                )
                            nc.scalar.activation(
                                em[:], sp[:], AF.Exp,
                                bias=(bias_mask[:] if diag else bias_zero[:]),
                                scale=0.125,
                            )
                        elif t < NPE:
                            nc.tensor.matmul(
                                sp[:],
                                kT[jr:jr + DK, jt, t * P:(t + 1) * P],
                                qT[jr:jr + DK, jt, ssl],
                                start=True, stop=False,
                            )
                            nc.tensor.matmul(
                                sp[:], idsc[:], keep_tiles[t][:],
                                start=False, stop=True,
                            )
                            nc.scalar.activation(
                                em[:], sp[:], AF.Exp,
                                bias=bias_mask[:], scale=0.125,
                            )
                        else:
                            nc.tensor.matmul(
                                sp[:],
                                kT[jr:jr + DK, jt, t * P:(t + 1) * P],
                                qT[jr:jr + DK, jt, ssl],
                                start=True, stop=True,
                            )
                            etmp = sb.tile([P, SBW], BF16, tag="etmp", bufs=4)
                            nc.scalar.activation(
                                etmp[:], sp[:], AF.Exp,
                                bias=bias_zero[:], scale=0.125,
                            )
                            nc.vector.tensor_mul(em[:], etmp[:], keep_tiles[t][:])
                        em_tiles.append(em)
                        if t >= LAG:
                            ctx_mm(t - LAG, last=nt_here - 1)
                        if t == FIN_T and pending_ctx[0] is not None:
                            pending_ctx[0]()
                            pending_ctx[0] = None
                        if t == FIN_T2 and pending_at[0] is not None:
                            pending_at[0]()
                            pending_at[0] = None
                    for t in range(max(0, nt_here - LAG), nt_here):
                        ctx_mm(t, last=nt_here - 1)
                    if pending_ctx[0] is not None:
                        pending_ctx[0]()
                        pending_ctx[0] = None
                    if pending_at[0] is not None:
                        pending_at[0]()
                        pending_at[0] = None
                    # softmax denominators sit in cps row DK; reciprocal as
                    # exp(-ln(x)) on the ScalarEngine (same table set as the
                    # softmax exp; DVE's iterative divide costs 2.7us here).
                    # The PE broadcast + normalize are deferred into the next
                    # head's score loop so the TensorEngine never waits.
                    lrow = sb.tile([1, SBW], F32, tag="lrow", bufs=4)
                    nc.scalar.activation(
                        lrow[:], cps[DK:DK + 1, :], AF.Ln,
                        bias=bias_zero[0:1, :], scale=1.0,
                    )
                    rrow = sb.tile([1, SBW], BF16, tag="rrow", bufs=4)
                    nc.scalar.activation(
                        rrow[:], lrow[:], AF.Exp,
                        bias=bias_zero[0:1, :], scale=-1.0,
                    )

                    rbc_box = {}

                    def fin_ctx(h=h, jt=jt, jr=jr, cps=cps, rrow=rrow,
                                box=rbc_box, ssl=ssl):
                        bps = ps.tile([P, SBW], F32, tag="ps_a", bufs=2)
                        nc.tensor.matmul(
                            bps[:], ones_col[:], rrow[:], start=True, stop=True
                        )
                        rbc = sb.tile([P, SBW], BF16, tag="rbc", bufs=2)
                        nc.vector.tensor_copy(rbc[:], bps[:])
                        box["rbc"] = rbc
                        nc.vector.tensor_mul(
                            ctxT[jr:jr + DK, jt, ssl], cps[0:DK, :], rbc[0:DK, :]
                        )

                    def fin_at(h=h, box=rbc_box, em_tiles=em_tiles,
                               nt_h=nt_here, sbk=sbk):
                        rbc = box["rbc"]
                        for t in range(nt_h):
                            at = sb.tile([P, SBW], BF16, tag="at", bufs=6)
                            nc.vector.tensor_mul(at[:], em_tiles[t][:], rbc[:])
                            nc.sync.dma_start(
                                attn_t[h, sbk, t * P:(t + 1) * P, :], at[:]
                            )

                    pending_ctx[0] = fin_ctx
                    pending_at[0] = fin_at
                # flush before the output projection: ctxT must be complete,
                # but the last head's attn-normalize can wait until after the
                # out-proj instructions are queued
                if pending_ctx[0] is not None:
                    pending_ctx[0]()
                    pending_ctx[0] = None
                last_at = pending_at[0]
                pending_at[0] = None
                if sbk < NSB - 1 and last_at is not None:
                    last_at()
                    last_at = None
                # ---- output projection for this s-block's row tiles ----
                for sti in range(SBW // P):
                    st = sbk * (SBW // P) + sti
                    for ob in range(D // SBW):
                        ops = ps.tile([P, SBW], F32, tag="ps_a", bufs=2)
                        for jt in range(NJT):
                            nc.tensor.matmul(
                                ops[:],
                                ctxT[:, jt, st * P:(st + 1) * P],
                                wo_sb[:, jt, ob * SBW:(ob + 1) * SBW],
                                start=(jt == 0), stop=(jt == NJT - 1),
                            )
                        ob_sb = sb.tile([P, SBW], BF16, tag="ob", bufs=4)
                        nc.vector.tensor_copy(ob_sb[:], ops[:])
                        nc.sync.dma_start(rs_ins[sbk][sti, ob], ob_sb[:])
                if last_at is not None:
                    last_at()
                # ---- reduce-scatter this chunk; rank r receives the reduced
                # rows [sbk*512 + r*128, +128) of this batch's output ----
                nc.gpsimd.collective_compute(
                    "ReduceScatter", ALU.add, replica_groups=REPLICA_GROUPS,
                    ins=[rs_ins[sbk].opt()], outs=[rs_outs[sbk].opt()],
                )
                # ---- residual + layernorm on the received 128 rows ----
                rsl = slice(sbk * P, (sbk + 1) * P)
                xr = sb.tile([P, D], BF16, tag="xr", bufs=2)
                for ob in range(D // SBW):
                    nc.sync.dma_start(
                        xr[:, ob * SBW:(ob + 1) * SBW], rs_outs[sbk][ob]
                    )
                qr = sb.tile([P, D], F32, tag="qr", bufs=2)
                nc.gpsimd.dma_start(qr[:], qres[rsl, :])
                x = sb.tile([P, D], F32, tag="x", bufs=2)
                nc.vector.tensor_add(x[:], xr[:], qr[:])
                stat = sb.tile([P, 8], F32, tag="stat", bufs=4)
                nc.vector.tensor_reduce(stat[:, 0:1], x[:], axis=AX.X, op=ALU.add)
                nc.vector.tensor_scalar_mul(stat[:, 1:2], stat[:, 0:1], -1.0 / D)
                xc = sb.tile([P, D], F32, tag="xc", bufs=2)
                nc.vector.tensor_scalar_add(xc[:], x[:], stat[:, 1:2])
                sqd = sb.tile([P, D], BF16, tag="sqd", bufs=2)
                nc.scalar.activation(sqd[:], xc[:], AF.Square, accum_out=stat[:, 2:3])
                nc.vector.tensor_scalar(
                    stat[:, 3:4], stat[:, 2:3], 1.0 / D, EPS, ALU.mult, ALU.add
                )
                # 1/sqrt(v) = exp(-0.5*ln(v)) -- stays in the exp/ln table set
                nc.scalar.activation(
                    stat[:, 4:5], stat[:, 3:4], AF.Ln, bias=bias_zero[:], scale=1.0
                )
                nc.scalar.activation(
                    stat[:, 5:6], stat[:, 4:5], AF.Exp, bias=bias_zero[:], scale=-0.5
                )
                y = sb.tile([P, D], F32, tag="y", bufs=2)
                nc.vector.scalar_tensor_tensor(
                    y[:], xc[:], stat[:, 5:6], gam_sb[:], op0=ALU.mult, op1=ALU.mult
                )
                nc.vector.tensor_add(y[:], y[:], bet_sb[:])
                nc.sync.dma_start(out_chunk[rsl, :], y[:])

    nc.compile()
    return nc


_PROGRAMS = {}


def _get_program(causal):
    if causal not in _PROGRAMS:
        _PROGRAMS[causal] = build_program(causal)
    return _PROGRAMS[causal]


def _core_rows(r):
    """Global row indices (within a batch) that core with group-rank r
    layernorms, in out_chunk order: 4 pieces of 128 rows, piece sbk holding
    rows [sbk*512 + r*128, +128)."""
    idx = []
    for sbk in range(NSB):
        start = sbk * SBW + r * P
        idx.extend(range(start, start + P))
    return np.asarray(idx)


def _build_in_maps(Q, K, V, attn_mask, Wq, bq, Wk, bk, Wv, bv, Wo, bo, gamma, beta):
    bf16 = mybir.dt.np(BF16)
    f32 = np.float32
    Q = np.asarray(Q, f32)
    K = np.asarray(K, f32)
    V = np.asarray(V, f32)
    mask = np.asarray(attn_mask, bool)
    Wq, Wk, Wv, Wo = (np.asarray(w, f32) for w in (Wq, Wk, Wv, Wo))
    bq, bk, bv, bo = (np.asarray(b_, f32) for b_ in (bq, bk, bv, bo))
    gamma = np.asarray(gamma, f32)
    beta = np.asarray(beta, f32)

    causal = bool(
        np.array_equal(
            mask, np.broadcast_to(np.triu(np.ones((S, S), bool), 1), (B, S, S))
        )
    )

    def _tile_xt(x):
        # [S, D] -> XT [D, S] -> [NSB, NKT, P, SBW]
        xt = x.T.astype(bf16)
        return np.ascontiguousarray(
            xt.reshape(NKT, P, NSB, SBW).transpose(2, 0, 1, 3)
        )

    qt_b = [_tile_xt(Q[b]) for b in range(B)]
    kt_b = [_tile_xt(K[b]) for b in range(B)]
    vt_b = [_tile_xt(V[b]) for b in range(B)]
    # keep[sb, t, s] = (~mask[b])[sb*SBW+s, t]
    keep_b = [
        np.ascontiguousarray(
            (~mask[b]).T.astype(bf16).reshape(S, NSB, SBW).transpose(1, 0, 2)
        )
        for b in range(B)
    ]
    gam_r = np.ascontiguousarray(np.broadcast_to(gamma, (P, D))).astype(f32)
    bet_r = np.ascontiguousarray(np.broadcast_to(beta, (P, D))).astype(f32)

    in_maps = []
    for c in range(NCORES):
        b, r = c // GROUP, c % GROUP
        j0 = r * JC
        rows = _core_rows(r)
        in_maps.append({
            "qt": qt_b[b], "kt": kt_b[b], "vt": vt_b[b], "keep": keep_b[b],
            "wqt": Wq[j0:j0 + JC, :].T.astype(bf16),
            "wkt": Wk[j0:j0 + JC, :].T.astype(bf16),
            "wvt": Wv[j0:j0 + JC, :].T.astype(bf16),
            "bqs": bq[j0:j0 + JC].reshape(JC, 1).astype(f32),
            "bks": bk[j0:j0 + JC].reshape(JC, 1).astype(f32),
            "bvs": bv[j0:j0 + JC].reshape(JC, 1).astype(f32),
            "wot": Wo[:, j0:j0 + JC].T.astype(bf16),
            "qres": (Q[b, rows, :] + bo).astype(f32),
            "gam": gam_r, "bet": bet_r,
        })
    return in_maps, causal


def _assemble(results):
    f32 = np.float32
    out = np.empty((B, S, D), f32)
    attn = np.empty((B, H, S, S), f32)
    for c in range(NCORES):
        b, r = c // GROUP, c % GROUP
        out[b, _core_rows(r)] = np.asarray(results[c]["out_chunk"], f32)
        a = np.asarray(results[c]["attn_t"]).astype(f32)  # [NH, sb, t, s]
        # attn[b, h, sb*SBW+s, t] = a[hl, sb, t, s]
        attn[b, r * NH:(r + 1) * NH] = (
            a.transpose(0, 1, 3, 2).reshape(NH, S, S)
        )
    return out, attn


def _run(in_maps, causal, trace=False, **kwargs):
    nc = _get_program(causal)
    return run_bass_kernel_spmd(
        nc, in_maps, core_ids=list(range(NCORES)), trace=trace, **kwargs
    )


def kernel(**inputs):
    in_maps, causal = _build_in_maps(**inputs)
    res = _run(in_maps, causal, trace=False)
    return _assemble(res.results)


def _enable_axon_ntff_hook():
    """Restore the NTFF profile hook that trn_boot could not register
    (antenv.axon_hooks is absent from this image) and skip the shared-bucket
    artifact upload. Only used by the local profiling harness."""
    import types

    import antenv
    import concourse.bass_utils as bu

    if "antenv.axon_hooks" not in sys.modules:
        mod = types.ModuleType("antenv.axon_hooks")
        mod._hook = None
        mod.set_axon_ntff_profile_hook = lambda h: setattr(mod, "_hook", h)
        mod.get_axon_ntff_profile_hook = lambda: mod._hook
        sys.modules["antenv.axon_hooks"] = mod
        antenv.axon_hooks = mod
        from trn_agent_boot.trn_boot import _ntff_profile_via_ctypes

        mod._hook = _ntff_profile_via_ctypes("/opt/axon/libaxon_pjrt.so")
    bu.upload_artifacts = lambda tmpdir: tmpdir


def kernel_profiled(tmpdir=None, **inputs):
    """Like kernel() but also returns neuron-profile exec time in ns."""
    _enable_axon_ntff_hook()
    in_maps, causal = _build_in_maps(**inputs)
    res = _run(in_maps, causal, trace=True, tmpdir=tmpdir)
    return _assemble(res.results), res.exec_time_ns
